# revision 4
# baseline (speedup 1.0000x reference)
"""Trainium2 Bass kernel for nn_HNetEnergyViaBoolWeights (v3.2).

Decomposition (see kernel.py / kernel2.py history):
    energies = K_bias + Wnode @ act + sum_e coeff_ab[c,e] * (a_e AND b_e)
with coeff_ab in {-1,0,1}.  The AND plane operands are nibble-encoded:
2 points per fp8 byte in the mantissa (byte = 0x38 | b0 | b1<<1), AND
of encodings = encoding of AND; two fp8 matmul "variants" (raw and
>>1-shifted) give an invertible affine mix of the per-point energies,
unmixed exactly on the host (E1 = 128*T1 - 12*S, E0 = 8*(T0-S) - 2*E1).

Edge streaming: per core the 6250-edge multigraph is capped to degree
<=4 and Euler-trail oriented, so every node has at most 2 a-uses and 2
b-uses.  Clean edges (~6140) occupy slots in walk order; each stream's
table rows are written in slot order (a node's row appears at most
twice per table - bounded replication, same class as replicating
tables across cores).  The clean region is therefore read with
SEQUENTIAL transfers in "quad" layout:

    slot s -> descriptor j = s//16 (lane k = s%16)
    desc j -> partition j%128, desc-chunk d = j//128  (DQ=3 chunks)

d-chunk 0 arrives via plain dma_start (sync engine for the A stream,
scalar for B); chunks 1 and 2 via 4KB-descriptor SWDGE gathers with
sequential indices on queues 0-3.  The ~110 repeat/overflow edges
("singles") are real 256B-descriptor gathers split across the 4
queues.  Matmuls consume the quad layout with k-sliced DoubleRows.

Host postprocess: sum 8 partial (64,1024) outputs, unmix T0/T1, add
K_bias, subtract the global min.
"""

import numpy as np
import ml_dtypes

N_PTS = 512
N_NODES = 10000
N_EDGES = 50000
N_CMP = 64
N_CORES = 8
EDGES_PER_CORE = N_EDGES // N_CORES
CK = 48                                  # clean chunks of 128 slots
SK = 2                                   # single chunks
NC_CAP = CK * 128                        # 6144
NS_CAP = SK * 128                        # 256
NDESC = CK * 8                           # 384 quad descriptors / stream
DQ = NDESC // 128                        # 3 desc-chunks
H_ROWS = 16160                           # table height (1010 groups)
NB = N_PTS // 2                          # 256B per packed row
V_CHUNKS = 10
NODES_PER_CORE = N_NODES // N_CORES
F8 = ml_dtypes.float8_e4m3

_compiled = None


def _build_bass(loop_iters=0):
    import concourse.mybir as mybir
    import concourse.tile as tile
    from concourse import bacc
    from concourse.library_config import mlp
    from contextlib import nullcontext

    dt = mybir.dt
    nc = bacc.Bacc("TRN2", target_bir_lowering=False, debug=False,
                   num_devices=N_CORES, num_swdge_queues=4)

    tabA_hw = nc.dram_tensor("tabA_hw", [128, 16 * NB], dt.float8e4,
                             kind="ExternalInput")
    tabB_hw = nc.dram_tensor("tabB_hw", [128, 16 * NB], dt.float8e4,
                             kind="ExternalInput")
    tabQA = nc.dram_tensor("tabQA", [H_ROWS // 16, 16 * NB], dt.float8e4,
                           kind="ExternalInput")
    tabQB = nc.dram_tensor("tabQB", [H_ROWS // 16, 16 * NB], dt.float8e4,
                           kind="ExternalInput")
    tabSA = nc.dram_tensor("tabSA", [H_ROWS, NB], dt.float8e4,
                           kind="ExternalInput")
    tabSB = nc.dram_tensor("tabSB", [H_ROWS, NB], dt.float8e4,
                           kind="ExternalInput")
    idx_qa = nc.dram_tensor("idx_qa", [128, 16], dt.int16,
                            kind="ExternalInput")
    idx_qb = nc.dram_tensor("idx_qb", [128, 16], dt.int16,
                            kind="ExternalInput")
    idx_sa = nc.dram_tensor("idx_sa", [128, NS_CAP // 16], dt.int16,
                            kind="ExternalInput")
    idx_sb = nc.dram_tensor("idx_sb", [128, NS_CAP // 16], dt.int16,
                            kind="ExternalInput")
    wqq = nc.dram_tensor("wqq", [128, DQ, 16, N_CMP], dt.float8e4,
                         kind="ExternalInput")
    wqs = nc.dram_tensor("wqs", [128, SK, N_CMP], dt.float8e4,
                         kind="ExternalInput")
    wnd = nc.dram_tensor("wnd", [128, 2, V_CHUNKS, N_CMP], dt.float8e4,
                         kind="ExternalInput")
    acts_nd = nc.dram_tensor("acts_nd", [128, V_CHUNKS, NB], dt.float8e4,
                             kind="ExternalInput")
    partial = nc.dram_tensor("partial", [N_CMP, 2 * N_PTS], dt.float32,
                             kind="ExternalOutput")

    with tile.TileContext(nc) as tc:
        with tc.tile_pool(name="sbuf", bufs=1) as pool, \
             tc.tile_pool(name="psum", bufs=1, space="PSUM") as psum_pool:
            nc.gpsimd.load_library(mlp)
            loop_cm = tc.For_i(0, loop_iters, 1) if loop_iters else nullcontext()
            with loop_cm:
                idx_qa_sb = pool.tile([128, 16], dt.int16, tag="idx_qa_sb")
                idx_qb_sb = pool.tile([128, 16], dt.int16, tag="idx_qb_sb")
                idx_sa_sb = pool.tile([128, NS_CAP // 16], dt.int16,
                                      tag="idx_sa_sb")
                idx_sb_sb = pool.tile([128, NS_CAP // 16], dt.int16,
                                      tag="idx_sb_sb")
                wqq_sb = pool.tile([128, DQ, 16, N_CMP], dt.float8e4,
                                   tag="wqq_sb")
                wqs_sb = pool.tile([128, SK, N_CMP], dt.float8e4,
                                   tag="wqs_sb")
                wnd_sb = pool.tile([128, 2, V_CHUNKS, N_CMP], dt.float8e4,
                                   tag="wnd_sb")
                actnd_sb = pool.tile([128, V_CHUNKS, 2, NB], dt.float8e4,
                                     tag="actnd_sb")
                gaq = pool.tile([128, DQ, 16 * NB], dt.float8e4, tag="gaq")
                gbq = pool.tile([128, DQ, 16 * NB], dt.float8e4, tag="gbq")
                gas = pool.tile([128, SK, NB], dt.float8e4, tag="gas")
                gbs = pool.tile([128, SK, NB], dt.float8e4, tag="gbs")
                gvq = pool.tile([128, DQ, 16, 2 * NB], dt.float8e4, tag="gvq")
                gvs = pool.tile([128, SK, 2 * NB], dt.float8e4, tag="gvs")

                # small idx loads first
                nc.sync.dma_start(idx_qa_sb[:], idx_qa[:])
                nc.sync.dma_start(idx_qb_sb[:], idx_qb[:])
                nc.sync.dma_start(idx_sa_sb[:], idx_sa[:])
                nc.sync.dma_start(idx_sb_sb[:], idx_sb[:])

                # SWDGE quad gathers: d-chunks 1, 2 (sequential indices)
                for (dst, src, idxs, d, q) in (
                        (gaq, tabQA, idx_qa_sb, 1, 0),
                        (gbq, tabQB, idx_qb_sb, 1, 1),
                        (gaq, tabQA, idx_qa_sb, 2, 2),
                        (gbq, tabQB, idx_qb_sb, 2, 3)):
                    nc.gpsimd.dma_gather(
                        dst[:, d:d + 1, :], src[:],
                        idxs[:, (d - 1) * 8:d * 8],
                        128, 128, 16 * NB,
                        single_packet=True, queue_num=q)
                # singles gathers: one 128-idx call per chunk, spread
                # over q2/q3 (quad-d2 queues are the lighter ones)
                for (st, h, q) in ((0, 0, 0), (1, 0, 1), (0, 1, 2), (1, 1, 3)):
                    idxs = idx_sa_sb if st == 0 else idx_sb_sb
                    dst = gas if st == 0 else gbs
                    src = tabSA if st == 0 else tabSB
                    nc.gpsimd.dma_gather(
                        dst[:, h:h + 1, :], src[:],
                        idxs[:, h * 8:(h + 1) * 8],
                        128, 128, NB,
                        single_packet=True, queue_num=q)

                # big HWDGE reads: tables first so the AND pipeline starts
                # early; weights chunk-split behind them
                nc.sync.dma_start(gaq[:, 0, :], tabA_hw[:])
                nc.scalar.dma_start(gbq[:, 0, :], tabB_hw[:])
                for d in range(DQ):
                    nc.sync.dma_start(wqq_sb[:, d, :, :], wqq[:, d, :, :])
                nc.scalar.dma_start(wqs_sb[:], wqs[:])
                nc.scalar.dma_start(actnd_sb[:, :, 0, :], acts_nd[:])
                nc.scalar.dma_start(wnd_sb[:], wnd[:])
                # node-plane shift variant on device
                nc.vector.tensor_scalar(
                    actnd_sb[:, :, 1, :].bitcast(dt.uint16),
                    actnd_sb[:, :, 0, :].bitcast(dt.uint16),
                    1, 0x7F7F,
                    op0=mybir.AluOpType.logical_shift_right,
                    op1=mybir.AluOpType.bitwise_and)

                acc_ab = psum_pool.tile([N_CMP, N_PTS], dt.float32,
                                        tag="acc_ab")
                acc_nd = psum_pool.tile([N_CMP, N_PTS], dt.float32,
                                        tag="acc_nd")

                # quad region: AND + shift + k-sliced DoubleRows per d-chunk
                for d in range(DQ):
                    nc.vector.tensor_tensor(
                        gvq[:, d, :, :NB].bitcast(dt.uint16),
                        gaq[:, d, :].bitcast(dt.uint16),
                        gbq[:, d, :].bitcast(dt.uint16),
                        op=mybir.AluOpType.bitwise_and)
                    nc.vector.tensor_scalar(
                        gvq[:, d, :, NB:].bitcast(dt.uint16),
                        gvq[:, d, :, :NB].bitcast(dt.uint16),
                        1, 0x7F7F,
                        op0=mybir.AluOpType.logical_shift_right,
                        op1=mybir.AluOpType.bitwise_and)
                    for k in range(0, 16, 2):
                        nc.tensor.matmul(
                            acc_ab[:], wqq_sb[:, d, k:k + 2, :],
                            gvq[:, d, k:k + 2, :],
                            start=(d == 0 and k == 0), stop=False,
                            perf_mode=mybir.MatmulPerfMode.DoubleRow)

                # singles region
                nc.vector.tensor_tensor(
                    gvs[:, :, :NB].bitcast(dt.uint16),
                    gas[:].bitcast(dt.uint16),
                    gbs[:].bitcast(dt.uint16),
                    op=mybir.AluOpType.bitwise_and)
                nc.vector.tensor_scalar(
                    gvs[:, :, NB:].bitcast(dt.uint16),
                    gvs[:, :, :NB].bitcast(dt.uint16),
                    1, 0x7F7F,
                    op0=mybir.AluOpType.logical_shift_right,
                    op1=mybir.AluOpType.bitwise_and)
                nc.tensor.matmul(
                    acc_ab[:], wqs_sb[:, 0:2, :], gvs[:, 0:2, :],
                    start=False, stop=True,
                    perf_mode=mybir.MatmulPerfMode.DoubleRow)

                # node-plane matmuls last (their data arrives mid-stream)
                for h in (0, 1):
                    for v in range(0, V_CHUNKS, 2):
                        nc.tensor.matmul(
                            acc_nd[:], wnd_sb[:, h, v:v + 2, :],
                            actnd_sb[:, v:v + 2, :, :],
                            start=(h == 0 and v == 0),
                            stop=(h == 1 and v == V_CHUNKS - 2),
                            perf_mode=mybir.MatmulPerfMode.DoubleRow)

                out_sb = pool.tile([N_CMP, 2 * N_PTS], dt.float32,
                                   tag="out_sb")
                nc.scalar.copy(out_sb[:, :N_PTS], acc_ab[:])
                nc.scalar.copy(out_sb[:, N_PTS:], acc_nd[:])
                nc.scalar.dma_start(partial[:], out_sb[:])

    nc.compile()
    return nc


def _get_compiled():
    global _compiled
    if _compiled is None:
        _compiled = _build_bass()
    return _compiled


def _wrap_idx(idx_stream, n, fill=0):
    pad = np.full(n, fill, np.int16)
    pad[:len(idx_stream)] = np.asarray(idx_stream, np.int16)
    return np.tile(pad.reshape(-1, 16).T.copy(), (8, 1))


def _plan_core(E):
    """Cap the multigraph at degree 4, Euler-orient the kept subgraph so
    every node has <=2 a-uses and <=2 b-uses.  Returns (clean, single)
    lists of (a, b, eid); len(clean) <= NC_CAP."""
    deg = np.zeros(N_NODES, np.int32)
    np.add.at(deg, E[:, 0], 1)
    np.add.at(deg, E[:, 1], 1)
    keysum = deg[E[:, 0]] + deg[E[:, 1]]
    uses = np.zeros(N_NODES, np.int32)
    keep = np.zeros(len(E), bool)
    for i in np.argsort(keysum, kind="stable"):
        u, v = int(E[i, 0]), int(E[i, 1])
        if u == v:
            if uses[u] <= 2:
                uses[u] += 2
                keep[i] = True
        elif uses[u] < 4 and uses[v] < 4:
            uses[u] += 1
            uses[v] += 1
            keep[i] = True
    kept = np.nonzero(keep)[0]

    adj, ptr = {}, {}
    for eid in kept:
        u, v = int(E[eid, 0]), int(E[eid, 1])
        adj.setdefault(u, []).append((eid, v))
        adj.setdefault(v, []).append((eid, u))
    visited = set()
    clean = []

    def walk(start):
        node = start
        while True:
            lst = adj[node]
            p = ptr.get(node, 0)
            while p < len(lst) and lst[p][0] in visited:
                p += 1
            ptr[node] = p
            if p >= len(lst):
                return
            eid, other = lst[p]
            visited.add(eid)
            clean.append((node, other, eid))
            node = other

    for n in [n for n, lst in adj.items() if len(lst) % 2 == 1]:
        walk(n)
    for n in adj:
        walk(n)
    assert len(clean) == len(kept)
    single = [(int(E[i, 0]), int(E[i, 1]), i) for i in np.nonzero(~keep)[0]]
    if len(clean) > NC_CAP:                 # overflow -> singles
        for a, b, eid in clean[NC_CAP:]:
            single.append((a, b, eid))
        clean = clean[:NC_CAP]
    assert len(single) <= NS_CAP, len(single)
    return clean, single


def prepare_in_maps(node_activations, binarized_learned, edge_endnode_idx):
    act = np.asarray(node_activations).astype(np.uint8)
    act_t = np.ascontiguousarray(act.T)
    W = np.asarray(binarized_learned)
    idx = np.asarray(edge_endnode_idx)

    P = W.reshape(N_CMP, 5, N_EDGES)
    P1, P2, P3, P4 = P[:, 1], P[:, 2], P[:, 3], P[:, 4]
    K_bias = P1.sum(axis=1, dtype=np.float64).astype(np.float32)
    coeff_a = P3 - P1
    coeff_b = P2 - P1
    coeff_ab = P4 - P3 - P2 + P1

    Wnode = np.zeros((N_NODES, N_CMP), np.float32)
    np.add.at(Wnode, idx[:, 0], coeff_a.T)
    np.add.at(Wnode, idx[:, 1], coeff_b.T)

    enc_all = (0x38 | (act_t[:, 0::2] | (act_t[:, 1::2] << 1))).astype(np.uint8)

    in_maps, S_list, SW_list = [], [], []
    for s in range(N_CORES):
        sl = slice(s * EDGES_PER_CORE, (s + 1) * EDGES_PER_CORE)
        E = idx[sl]
        cab = coeff_ab[:, sl]
        clean, single = _plan_core(E)

        # tables: slot rows (with 0x38 filler padding), then first-copy
        # rows for nodes without a clean use on that side
        def build_table(nodes_in_slot_order):
            tab = np.full((H_ROWS, NB), 0x38, np.uint8)
            row_of = np.full(N_NODES, -1, np.int32)
            for r, n in enumerate(nodes_in_slot_order):
                tab[r] = enc_all[n]
                if row_of[n] < 0:
                    row_of[n] = r
            nxt = NC_CAP
            for n in range(N_NODES):
                if row_of[n] < 0:
                    tab[nxt] = enc_all[n]
                    row_of[n] = nxt
                    nxt += 1
            assert nxt <= H_ROWS
            return tab, row_of

        tabA, rowA = build_table([a for a, _, _ in clean])
        tabB, rowB = build_table([b for _, b, _ in clean])

        coeffs_q = np.zeros((N_CMP, NC_CAP), np.float32)
        for slot, (_, _, eid) in enumerate(clean):
            coeffs_q[:, slot] = cab[:, eid]
        coeffs_s = np.zeros((N_CMP, NS_CAP), np.float32)
        for j, (_, _, eid) in enumerate(single):
            coeffs_s[:, j] = cab[:, eid]
        S_list.append(coeffs_q.sum(axis=1) + coeffs_s.sum(axis=1))

        # wqq[p, d, k, c] = coeffs_q[c, ((d*128+p)*16 + k)]
        wqq_arr = np.ascontiguousarray(
            coeffs_q.reshape(N_CMP, DQ, 128, 16).transpose(2, 1, 3, 0)
        ).astype(F8)
        wqs_arr = np.ascontiguousarray(
            coeffs_s.reshape(N_CMP, SK, 128).transpose(2, 1, 0)).astype(F8)

        # HWDGE portion: descs [0,128): tab_hw[p, :] = rows[16p:16p+16]
        hwA = np.ascontiguousarray(tabA[:2048].reshape(128, 16 * NB))
        hwB = np.ascontiguousarray(tabB[:2048].reshape(128, 16 * NB))

        qidx = np.arange(128, NDESC, dtype=np.int16)   # groups 128..383
        ia_q = _wrap_idx(qidx, 256)

        ia_s = _wrap_idx([rowA[a] for a, _, _ in single], NS_CAP)
        ib_s = _wrap_idx([rowB[b] for _, b, _ in single], NS_CAP)

        vsl = slice(s * NODES_PER_CORE, (s + 1) * NODES_PER_CORE)
        wnd_pad = np.zeros((V_CHUNKS * 128, N_CMP), np.float32)
        wnd_pad[:NODES_PER_CORE] = Wnode[vsl]
        hi = np.round(wnd_pad / 16.0)
        lo = wnd_pad - 16.0 * hi
        wnd_arr = np.ascontiguousarray(
            np.stack([lo, 16.0 * hi], axis=0)
            .reshape(2, V_CHUNKS, 128, N_CMP).transpose(2, 0, 1, 3)).astype(F8)
        # nibble-packed node activations (same 0x38-mantissa encoding);
        # pad nodes encode as act=0 pairs (0x38) with zero weights
        and_pad = np.full((V_CHUNKS * 128, NB), 0x38, np.uint8)
        and_pad[:NODES_PER_CORE] = enc_all[vsl]
        actnd = np.ascontiguousarray(
            and_pad.reshape(V_CHUNKS, 128, NB).transpose(1, 0, 2)).view(F8)
        SW_list.append(wnd_pad.sum(axis=0))

        in_maps.append({
            "tabA_hw": hwA.view(F8), "tabB_hw": hwB.view(F8),
            "tabQA": np.ascontiguousarray(
                tabA.reshape(H_ROWS // 16, 16 * NB)).view(F8),
            "tabQB": np.ascontiguousarray(
                tabB.reshape(H_ROWS // 16, 16 * NB)).view(F8),
            "tabSA": tabA.view(F8), "tabSB": tabB.view(F8),
            "idx_qa": ia_q, "idx_qb": ia_q.copy(),
            "idx_sa": ia_s, "idx_sb": ib_s,
            "wqq": wqq_arr, "wqs": wqs_arr,
            "wnd": wnd_arr, "acts_nd": actnd,
        })
    return in_maps, K_bias, np.stack(S_list), np.stack(SW_list)


def _unmix(T0, T1, Sc):
    E1 = 128.0 * T1 - 12.0 * Sc
    E0 = 8.0 * (T0 - Sc) - 2.0 * E1
    E = np.empty((N_CMP, N_PTS))
    E[:, 0::2] = E0
    E[:, 1::2] = E1
    return E


def postprocess(results, K_bias, S, SW):
    total = np.zeros((N_CMP, N_PTS), np.float64)
    for c, r in enumerate(results):
        P = r["partial"].astype(np.float64)
        total += _unmix(P[:, :256], P[:, 256:512],
                        S[c].astype(np.float64)[:, None])
        total += _unmix(P[:, 512:768], P[:, 768:],
                        SW[c].astype(np.float64)[:, None])
    energies = total + K_bias.astype(np.float64)[:, None]
    out = energies.T - energies.min()
    return np.ascontiguousarray(out.astype(np.float32))


def kernel(node_activations, binarized_learned, edge_endnode_idx,
           _bass_kwargs=None):
    from concourse.bass_utils import run_bass_kernel_spmd

    nc = _get_compiled()
    in_maps, K_bias, S, SW = prepare_in_maps(
        node_activations, binarized_learned, edge_endnode_idx)
    res = run_bass_kernel_spmd(nc, in_maps, core_ids=list(range(N_CORES)),
                               **(_bass_kwargs or {}))
    out = postprocess(res.results, K_bias, S, SW)
    kernel.last_results = res
    return out
